# revision 30
# baseline (speedup 1.0000x reference)
"""CharRNN (128-layer stacked LSTM, H=64, T=128, B=1) on 8 Trainium2 cores.

Strategy: pipeline-parallel over layers (16 layers/core), wavefront ticks
inside each core. Per LSTM cell the two gate-half matvecs are computed as
  matmul(out=(128,1) psum column, lhsT=W_half^T (128x128, stationary, fp16),
         rhs=[y; h] (128,1))
so the 16 cells of a tick form (128,16) gate tiles with gates on partitions
and cells on the free dim. Optimizations over the v0 kernel:
  - bias folded into PSUM via a seed matmul (lhsT=bias rows, rhs=identity)
    so activations read gates straight from PSUM (no bias add on DVE);
  - gate halves A=[f;i] / B=[o;g] are separate PSUM tiles and bursts, so
    sigmoid(A) overlaps the B matmul burst;
  - pipeline skew S: round r consumes the AllGather issued at round r-S,
    hiding collective latency + slot-select behind a full round of compute;
  - engine split: ACT does inject+activations, DVE does products/copies,
    GPSIMD only slot-selects + collectives; per-round work is hoisted a
    round early so round boundaries cost nothing on the critical path;
  - state zeroing for pipeline fill is folded into the last-tick h write
    (scalar_tensor_tensor with a per-core mask column).
"""

import sys

sys.path.insert(0, "/opt/trn_rl_repo")

from contextlib import ExitStack

import numpy as np

import concourse.bass as bass
import concourse.mybir as mybir
from concourse import bacc, tile
from concourse.bass_utils import run_bass_kernel_spmd

F32 = mybir.dt.float32
HDT = mybir.dt.float16
HDT_NP = np.float16

H = 64
NL = 128
T = 128
V = 35
NCORE = 8
LPC = NL // NCORE          # 16 layers per core
C = 2                      # timesteps per pipeline round
S = 3                      # skew: round r consumes the AllGather of round r-S
R = T // C
ROUNDS = R + S * (NCORE - 1)
FILL_MAX = S * (NCORE - 1)  # rounds <= S*k have core k's state zeroed

_CACHE = {}

SIG = mybir.ActivationFunctionType.Sigmoid
TANH = mybir.ActivationFunctionType.Tanh
MUL_OP = mybir.AluOpType.mult
ADD_OP = mybir.AluOpType.add


def _build():
    nc = bacc.Bacc()

    wts_d = nc.declare_dram_parameter("wts", [128, 2 * LPC * 128], HDT, isOutput=False)
    bT_d = nc.declare_dram_parameter("bT", [LPC, 2 * 128], F32, isOutput=False)
    idl_d = nc.declare_dram_parameter("idl", [LPC, LPC], F32, isOutput=False)
    selm_d = nc.declare_dram_parameter("selm", [64, C * NCORE], F32, isOutput=False)
    xest_d = nc.declare_dram_parameter("xest", [64, ROUNDS * C], F32, isOutput=False)
    mcol_d = nc.declare_dram_parameter("mcol", [64, ROUNDS + 1], F32, isOutput=False)
    fcol_d = nc.declare_dram_parameter("fcol", [64, ROUNDS], HDT, isOutput=False)
    ones_d = nc.declare_dram_parameter("onesv", [128, V], F32, isOutput=False)
    vct_d = nc.declare_dram_parameter("vct", [128, 1], F32, isOutput=False)
    wfc_d = nc.declare_dram_parameter("wfct", [64, V], HDT, isOutput=False)
    bfc_d = nc.declare_dram_parameter("bfc", [V, 1], F32, isOutput=False)
    iot_d = nc.declare_dram_parameter("iotar", [128, V], F32, isOutput=False)
    idn_d = nc.declare_dram_parameter("idn", [V, V], F32, isOutput=False)
    aginit_d = nc.declare_dram_parameter("aginit", [NCORE, 64, C], HDT, isOutput=False)
    out_d = nc.declare_dram_parameter("out_idx", [128, 1], F32, isOutput=True)

    NCC = ROUNDS - S  # collectives actually consumed
    ccin = [nc.dram_tensor(f"ccin{r}", [64, C], HDT) for r in range(NCC)]
    agout = [
        nc.dram_tensor(f"agout{r}", [NCORE, 64, C], HDT, addr_space="Shared")
        for r in range(NCC)
    ]
    hfin = nc.dram_tensor("hfin", [64, LPC], HDT)
    hfall = nc.dram_tensor("hfall", [NCORE, 64, LPC], HDT, addr_space="Shared")
    warm_in = nc.dram_tensor("warm_in", [64, C], HDT)
    warm_out = nc.dram_tensor("warm_out", [NCORE, 64, C], HDT, addr_space="Shared")

    groups = [list(range(NCORE))]

    with tile.TileContext(nc) as tc, ExitStack() as ctx:
        # warmup AllGather: starts collective-engine init at t=0 (it is
        # otherwise lazy, ~90us) and syncs the cores' CC pipelines before
        # round 0's collective lands on them
        const = ctx.enter_context(tc.tile_pool(name="const", bufs=1))
        # no input dependency: garbage data is fine, nothing consumes it
        nc.gpsimd.collective_compute(
            "AllGather", mybir.AluOpType.bypass, replica_groups=[list(range(NCORE))],
            ins=[warm_in[:]], outs=[warm_out[:]],
        )
        state = ctx.enter_context(tc.tile_pool(name="state", bufs=1))
        work = ctx.enter_context(tc.tile_pool(name="work", bufs=3))
        gpool = ctx.enter_context(tc.tile_pool(name="gpool", bufs=2, space="PSUM"))

        # ---- constants ----
        wsb = const.tile([128, 2 * LPC * 128], HDT)
        nc.sync.dma_start(out=wsb[:], in_=wts_d[:])
        wsb_v = wsb.rearrange("k (j m) -> k j m", m=128)
        bT = const.tile([LPC, 2 * 128], F32)
        nc.sync.dma_start(out=bT[:], in_=bT_d[:])
        idl = const.tile([LPC, LPC], F32)
        nc.sync.dma_start(out=idl[:], in_=idl_d[:])
        selm = const.tile([64, C * NCORE], F32)
        nc.sync.dma_start(out=selm[:], in_=selm_d[:])
        xest = const.tile([64, ROUNDS * C], F32)
        nc.sync.dma_start(out=xest[:], in_=xest_d[:])
        mcol = const.tile([64, ROUNDS + 1], F32)
        nc.sync.dma_start(out=mcol[:], in_=mcol_d[:])
        # fcol/fh live on partitions 64:128 so the scalar_tensor_tensor
        # accumulate shares its start partition with lhsT's h half
        fcolw = const.tile([128, ROUNDS], HDT)
        nc.sync.dma_start(out=fcolw[64:128, :], in_=fcol_d[:])
        onesv = const.tile([128, V], F32)
        nc.sync.dma_start(out=onesv[:], in_=ones_d[:])
        vct = const.tile([128, 1], F32)
        nc.sync.dma_start(out=vct[:], in_=vct_d[:])
        wfct = const.tile([64, V], HDT)
        nc.sync.dma_start(out=wfct[:], in_=wfc_d[:])
        bfct = const.tile([V, 1], F32)
        nc.sync.dma_start(out=bfct[:], in_=bfc_d[:])
        iotar = const.tile([128, V], F32)
        nc.sync.dma_start(out=iotar[:], in_=iot_d[:])
        idn = const.tile([V, V], F32)
        nc.sync.dma_start(out=idn[:], in_=idn_d[:])

        # ---- persistent state ----
        warm_act = state.tile([64, 1], F32)
        nc.vector.memset(warm_act[:], 0.0)
        nc.scalar.activation(warm_act[:], warm_act[:], SIG)
        nc.scalar.activation(warm_act[:], warm_act[:],
                             mybir.ActivationFunctionType.Exp)
        lhsT = state.tile([128, LPC], HDT)   # rows 0:64 = y inputs, 64:128 = h
        cst = state.tile([64, LPC], F32)     # cell state
        fhw = state.tile([128, LPC], HDT)    # final-h accumulator (rows 64:128)
        nc.vector.memset(lhsT[:], 0.0)
        nc.vector.memset(cst[:], 0.0)
        nc.vector.memset(fhw[:], 0.0)

        def issue_slot_dma(r):
            """Prefetch agout[r-S] into SBUF (Sync queue, off critical path)."""
            agprev = aginit_d if r < S else agout[r - S]
            slots = work.tile([64, NCORE * C], HDT, tag="slots")
            nc.sync.dma_start(
                out=slots.rearrange("p (s t) -> p s t", t=C),
                in_=agprev[:].rearrange("s p t -> p s t"),
            )
            return slots

        def issue_slot_select(r, slots):
            """inch(r) = sel(slots) + xest[r] (DVE, at round-r start)."""
            tmp8 = work.tile([64, C * NCORE], F32, tag="tmp8")
            nc.vector.tensor_mul(
                tmp8.rearrange("p (t s) -> p t s", s=NCORE),
                slots.rearrange("p (s t) -> p t s", t=C),
                selm.rearrange("p (t s) -> p t s", s=NCORE),
            )
            inch = work.tile([64, C], F32, tag="inch")
            nc.vector.tensor_reduce(
                out=inch[:],
                in_=tmp8.rearrange("p (t s) -> p t s", s=NCORE),
                axis=mybir.AxisListType.X,
                op=ADD_OP,
            )
            nc.vector.tensor_add(inch[:], inch[:], xest[:, r * C:(r + 1) * C])
            return inch

        slots_cur = issue_slot_dma(0)

        for r in range(ROUNDS):
            inch = issue_slot_select(r, slots_cur)
            # state masking for pipeline fill: cst zeroed at round start
            # (h/y were already masked by the previous round's last h write)
            if 0 < r <= FILL_MAX:
                nc.vector.tensor_scalar_mul(cst[:], cst[:], mcol[:, r:r + 1])

            outch = work.tile([64, C], HDT, tag="outch")

            for t in range(C):
                # prefetch next round's slots DMA late in this round
                if t == C - 1 and r + 1 < ROUNDS:
                    slots_next = issue_slot_dma(r + 1)

                # inject this tick's layer-0 input. Mid-round it hides on
                # ACT; at tick 0 it chains off the selects, so do it on DVE
                # to skip a cross-engine handoff.
                if t == 0:
                    nc.vector.tensor_copy(lhsT[0:64, 0:1], inch[:, t:t + 1])
                else:
                    nc.scalar.copy(lhsT[0:64, 0:1], inch[:, t:t + 1])

                gpA = gpool.tile([128, LPC], F32, tag="gA")
                gpB = gpool.tile([128, LPC], F32, tag="gB")
                # seed PSUM with the gate biases, then accumulate the matvecs
                nc.tensor.matmul(gpA[:], bT[:, 0:128], idl[:],
                                 start=True, stop=False, skip_group_check=True)
                nc.tensor.matmul(gpB[:], bT[:, 128:256], idl[:],
                                 start=True, stop=False, skip_group_check=True)
                for l in range(LPC):
                    nc.tensor.matmul(
                        gpA[:, l:l + 1], wsb_v[:, 2 * l, :], lhsT[:, l:l + 1],
                        start=False, stop=True, skip_group_check=True,
                    )
                for l in range(LPC):
                    nc.tensor.matmul(
                        gpB[:, l:l + 1], wsb_v[:, 2 * l + 1, :], lhsT[:, l:l + 1],
                        start=False, stop=True, skip_group_check=True,
                    )

                # gate row order (host-permuted): A = [f|i], B = [o|g]
                sgA = work.tile([128, LPC], F32, tag="sgA")
                sgB = work.tile([128, LPC], F32, tag="sgB")
                nc.scalar.activation(sgA[:], gpA[:], SIG)
                nc.scalar.activation(sgB[64:128, :], gpB[64:128, :], TANH)
                nc.scalar.activation(sgB[0:64, :], gpB[0:64, :], SIG)

                t1 = work.tile([64, LPC], F32, tag="t1")
                nc.vector.tensor_mul(t1[:], sgA[0:64, :], cst[:])
                t2 = work.tile([64, LPC], F32, tag="t2")
                nc.vector.tensor_mul(t2[:], sgA[64:128, :], sgB[64:128, :])
                nc.vector.tensor_add(cst[:], t1[:], t2[:])

                tct = work.tile([64, LPC], F32, tag="tct")
                nc.scalar.activation(tct[:], cst[:], TANH)

                # h = sigmoid(o) * tanh(c); on the round's last tick fold in
                # the next round's fill mask so downstream state starts at 0
                if t == C - 1 and r + 1 <= FILL_MAX:
                    nc.vector.tensor_scalar_mul(tct[:], tct[:], mcol[:, r + 1:r + 2])
                nc.vector.tensor_mul(lhsT[64:128, :], sgB[0:64, :], tct[:])
                nc.vector.tensor_copy(lhsT[0:64, 1:LPC], lhsT[64:128, 0:LPC - 1])
                nc.vector.tensor_copy(outch[:, t:t + 1], lhsT[64:128, LPC - 1:LPC])

            # accumulate the final h on the one round where it's this core's
            if r >= R - 1:
                nc.vector.scalar_tensor_tensor(
                    fhw[64:128, :], lhsT[64:128, :], fcolw[64:128, r:r + 1],
                    fhw[64:128, :], op0=MUL_OP, op1=ADD_OP,
                )

            if r + 1 < ROUNDS:
                slots_cur = slots_next

            # ship this round's boundary chunk (consumed at round r+S)
            if r < NCC:
                nc.scalar.dma_start(out=ccin[r][:], in_=outch[:])
                nc.gpsimd.collective_compute(
                    "AllGather", mybir.AluOpType.bypass, replica_groups=groups,
                    ins=[ccin[r][:]], outs=[agout[r][:]],
                )

        # ---- head: gather final h, logits, softmax over layers, argmax ----
        nc.sync.dma_start(out=hfin[:], in_=fhw[64:128, :])
        nc.gpsimd.collective_compute(
            "AllGather", mybir.AluOpType.bypass, replica_groups=groups,
            ins=[hfin[:]], outs=[hfall[:]],
        )
        HT = state.tile([64, NL], HDT)
        nc.sync.dma_start(
            out=HT.rearrange("p (s i) -> p s i", i=LPC),
            in_=hfall[:].rearrange("s p i -> p s i"),
        )
        logp = gpool.tile([V, NL], F32, tag="logp", bufs=1)
        nc.tensor.matmul(logp[:], wfct[:], HT[:], start=True, stop=True)
        logits = work.tile([V, NL], F32, tag="logits")
        nc.scalar.add(logits[:], logp[:], bfct[:, 0:1])

        mx = work.tile([V, 1], F32, tag="mx")
        nc.vector.tensor_reduce(
            out=mx[:], in_=logits[:], axis=mybir.AxisListType.X, op=mybir.AluOpType.max)
        nmx = work.tile([V, 1], F32, tag="nmx")
        nc.scalar.mul(nmx[:], mx[:], -1.0)
        ex = work.tile([V, NL], F32, tag="ex")
        nc.scalar.activation(
            ex[:], logits[:], mybir.ActivationFunctionType.Exp, bias=nmx[:, 0:1])
        sm = work.tile([V, 1], F32, tag="sm")
        nc.vector.tensor_reduce(
            out=sm[:], in_=ex[:], axis=mybir.AxisListType.X, op=ADD_OP)
        rsm = work.tile([V, 1], F32, tag="rsm")
        nc.vector.reciprocal(rsm[:], sm[:])
        probs = work.tile([V, NL], F32, tag="probs")
        nc.scalar.mul(probs[:], ex[:], rsm[:, 0:1])

        tp = gpool.tile([128, V], F32, tag="tp", bufs=1)
        nc.tensor.transpose(tp[:], probs[:], idn[:])
        m2 = work.tile([128, 1], F32, tag="m2")
        nc.vector.tensor_reduce(
            out=m2[:], in_=tp[:], axis=mybir.AxisListType.X, op=mybir.AluOpType.max)
        m2b = work.tile([128, V], F32, tag="m2b")
        nc.scalar.mul(m2b[:], onesv[:], m2[:, 0:1])
        eq = work.tile([128, V], F32, tag="eq")
        nc.vector.tensor_tensor(eq[:], tp[:], m2b[:], op=mybir.AluOpType.is_equal)
        val = work.tile([128, V], F32, tag="val")
        nc.vector.tensor_mul(val[:], eq[:], iotar[:])
        mr = work.tile([128, 1], F32, tag="mr")
        nc.vector.tensor_reduce(
            out=mr[:], in_=val[:], axis=mybir.AxisListType.X, op=mybir.AluOpType.max)
        idx = work.tile([128, 1], F32, tag="idx")
        nc.vector.tensor_sub(idx[:], vct[:], mr[:])
        nc.scalar.dma_start(out=out_d[:], in_=idx[:])

    nc.finalize()
    return nc


def _prep_in_maps(inputs):
    x = np.asarray(inputs["x"]).astype(np.int64)
    embed = np.asarray(inputs["embed"], dtype=np.float32)
    xe = embed[x, 0]  # (T,)

    Wih_full = np.zeros((NL, 4 * H, H), np.float32)
    Wih_full[0, :, 0] = np.asarray(inputs["Wih0"], np.float32)[:, 0]
    Wih_full[1:] = np.asarray(inputs["Wih"], np.float32)
    Whh_full = np.concatenate(
        [np.asarray(inputs["Whh0"], np.float32)[None],
         np.asarray(inputs["Whh"], np.float32)], axis=0)
    b_full = np.concatenate(
        [(np.asarray(inputs["bih0"], np.float32)
          + np.asarray(inputs["bhh0"], np.float32))[None],
         np.asarray(inputs["bih"], np.float32)
         + np.asarray(inputs["bhh"], np.float32)], axis=0)  # (NL, 256)

    Wcat = np.concatenate([Wih_full, Whh_full], axis=2)      # (NL, 256, 128)
    # permute pytorch gate order [i f g o] -> [f i o g] so the device layout
    # has half-A rows = [f; i] and half-B rows = [o; g]
    perm = np.r_[64:128, 0:64, 192:256, 128:192]
    Wcat = Wcat[:, perm, :]
    b_full = b_full[:, perm]
    lhsT_all = np.ascontiguousarray(np.transpose(Wcat, (0, 2, 1)))  # (NL,128,256)

    wfct = np.asarray(inputs["Wfc"], np.float32).T.astype(HDT_NP)  # (64, V)
    bfc = np.asarray(inputs["bfc"], np.float32).reshape(V, 1)
    iotar = np.broadcast_to(
        (V - np.arange(V, dtype=np.float32))[None, :], (128, V)).copy()
    idn = np.eye(V, dtype=np.float32)
    aginit = np.zeros((NCORE, 64, C), HDT_NP)

    in_maps = []
    for k in range(NCORE):
        lhsT_k = lhsT_all[k * LPC:(k + 1) * LPC]  # (LPC, 128, 256)
        wts = (lhsT_k.reshape(LPC, 128, 2, 128)
               .transpose(0, 2, 1, 3)
               .reshape(2 * LPC, 128, 128).astype(HDT_NP))
        wts = np.ascontiguousarray(wts.transpose(1, 0, 2)).reshape(128, 2 * LPC * 128)
        bT = np.ascontiguousarray(b_full[k * LPC:(k + 1) * LPC].astype(np.float32))  # (LPC, 256)

        selm = np.zeros((64, C, NCORE), np.float32)
        if k > 0:
            selm[:, :, k - 1] = 1.0
        xest = np.zeros((64, ROUNDS * C), np.float32)
        if k == 0:
            xest[0, :T] = xe
        mcol = np.ones((64, ROUNDS + 1), np.float32)
        mcol[:, :S * k + 1] = 0.0  # zero state at start of rounds <= S*k
        fcol = np.zeros((64, ROUNDS), HDT_NP)
        fcol[:, S * k + R - 1] = 1.0

        in_maps.append({
            "wts": wts,
            "bT": bT,
            "idl": np.eye(LPC, dtype=np.float32),
            "selm": selm.reshape(64, C * NCORE),
            "xest": xest,
            "mcol": mcol,
            "fcol": fcol,
            "onesv": np.ones((128, V), np.float32),
            "vct": np.full((128, 1), float(V), np.float32),
            "wfct": wfct,
            "bfc": bfc,
            "iotar": iotar,
            "idn": idn,
            "aginit": aginit,
        })
    return in_maps


def _run(inputs, trace=False):
    if "nc" not in _CACHE:
        _CACHE["nc"] = _build()
    nc = _CACHE["nc"]
    in_maps = _prep_in_maps(inputs)
    res = run_bass_kernel_spmd(nc, in_maps, list(range(NCORE)), trace=trace)
    out = np.asarray(res.results[0]["out_idx"], np.float32).reshape(NL)
    idx = np.rint(out).astype(np.int32)
    return idx, res


def kernel(**inputs) -> np.ndarray:
    idx, _ = _run(inputs, trace=False)
    return idx


# revision 31
# speedup vs baseline: 1.4425x; 1.4425x over previous
"""CharRNN (128-layer stacked LSTM, H=64, T=128, B=1) on 8 Trainium2 cores.

Strategy: pipeline-parallel over layers (16 layers/core), wavefront ticks
inside each core. Per LSTM cell the two gate-half matvecs are computed as
  matmul(out=(128,1) psum column, lhsT=W_half^T (128x128, stationary, fp16),
         rhs=[y; h] (128,1))
so the 16 cells of a tick form (128,16) gate tiles with gates on partitions
and cells on the free dim. Optimizations over the v0 kernel:
  - bias folded into PSUM via a seed matmul (lhsT=bias rows, rhs=identity)
    so activations read gates straight from PSUM (no bias add on DVE);
  - gate halves A=[f;i] / B=[o;g] are separate PSUM tiles and bursts, so
    sigmoid(A) overlaps the B matmul burst;
  - pipeline skew S: round r consumes the AllGather issued at round r-S,
    hiding collective latency + slot-select behind a full round of compute;
  - engine split: ACT does inject+activations, DVE does products/copies,
    GPSIMD only slot-selects + collectives; per-round work is hoisted a
    round early so round boundaries cost nothing on the critical path;
  - state zeroing for pipeline fill is folded into the last-tick h write
    (scalar_tensor_tensor with a per-core mask column).
"""

import sys

sys.path.insert(0, "/opt/trn_rl_repo")

from contextlib import ExitStack

import numpy as np

import concourse.bass as bass
import concourse.mybir as mybir
from concourse import bacc, tile
from concourse.bass_utils import run_bass_kernel_spmd

F32 = mybir.dt.float32
HDT = mybir.dt.float16
HDT_NP = np.float16

H = 64
NL = 128
T = 128
V = 35
NCORE = 8
LPC = NL // NCORE          # 16 layers per core
C = 4                      # timesteps per pipeline round
S = 2                      # skew: round r consumes the AllGather of round r-S
R = T // C
ROUNDS = R + S * (NCORE - 1)
FILL_MAX = S * (NCORE - 1)  # rounds <= S*k have core k's state zeroed

_CACHE = {}

SIG = mybir.ActivationFunctionType.Sigmoid
TANH = mybir.ActivationFunctionType.Tanh
MUL_OP = mybir.AluOpType.mult
ADD_OP = mybir.AluOpType.add


def _build():
    nc = bacc.Bacc()

    wts_d = nc.declare_dram_parameter("wts", [128, 2 * LPC * 128], HDT, isOutput=False)
    bT_d = nc.declare_dram_parameter("bT", [LPC, 2 * 128], F32, isOutput=False)
    idl_d = nc.declare_dram_parameter("idl", [LPC, LPC], F32, isOutput=False)
    selm_d = nc.declare_dram_parameter("selm", [64, C * NCORE], F32, isOutput=False)
    xest_d = nc.declare_dram_parameter("xest", [64, ROUNDS * C], F32, isOutput=False)
    mcol_d = nc.declare_dram_parameter("mcol", [64, ROUNDS + 1], F32, isOutput=False)
    fcol_d = nc.declare_dram_parameter("fcol", [64, ROUNDS], HDT, isOutput=False)
    ones_d = nc.declare_dram_parameter("onesv", [128, V], F32, isOutput=False)
    vct_d = nc.declare_dram_parameter("vct", [128, 1], F32, isOutput=False)
    wfc_d = nc.declare_dram_parameter("wfct", [64, V], HDT, isOutput=False)
    bfc_d = nc.declare_dram_parameter("bfc", [V, 1], F32, isOutput=False)
    iot_d = nc.declare_dram_parameter("iotar", [128, V], F32, isOutput=False)
    idn_d = nc.declare_dram_parameter("idn", [V, V], F32, isOutput=False)
    aginit_d = nc.declare_dram_parameter("aginit", [NCORE, 64, C], HDT, isOutput=False)
    out_d = nc.declare_dram_parameter("out_idx", [128, 1], F32, isOutput=True)

    NCC = ROUNDS - S  # collectives actually consumed
    ccin = [nc.dram_tensor(f"ccin{r}", [64, C], HDT) for r in range(NCC)]
    agout = [
        nc.dram_tensor(f"agout{r}", [NCORE, 64, C], HDT, addr_space="Shared")
        for r in range(NCC)
    ]
    hfin = nc.dram_tensor("hfin", [64, LPC], HDT)
    hfall = nc.dram_tensor("hfall", [NCORE, 64, LPC], HDT, addr_space="Shared")
    warm_in = nc.dram_tensor("warm_in", [64, C], HDT)
    warm_out = nc.dram_tensor("warm_out", [NCORE, 64, C], HDT, addr_space="Shared")

    groups = [list(range(NCORE))]

    with tile.TileContext(nc) as tc, ExitStack() as ctx:
        # warmup AllGather: starts collective-engine init at t=0 (it is
        # otherwise lazy, ~90us) and syncs the cores' CC pipelines before
        # round 0's collective lands on them
        const = ctx.enter_context(tc.tile_pool(name="const", bufs=1))
        # no input dependency: garbage data is fine, nothing consumes it
        nc.gpsimd.collective_compute(
            "AllGather", mybir.AluOpType.bypass, replica_groups=[list(range(NCORE))],
            ins=[warm_in[:]], outs=[warm_out[:]],
        )
        state = ctx.enter_context(tc.tile_pool(name="state", bufs=1))
        work = ctx.enter_context(tc.tile_pool(name="work", bufs=3))
        gpool = ctx.enter_context(tc.tile_pool(name="gpool", bufs=2, space="PSUM"))

        # ---- constants ----
        wsb = const.tile([128, 2 * LPC * 128], HDT)
        nc.sync.dma_start(out=wsb[:], in_=wts_d[:])
        wsb_v = wsb.rearrange("k (j m) -> k j m", m=128)
        bT = const.tile([LPC, 2 * 128], F32)
        nc.sync.dma_start(out=bT[:], in_=bT_d[:])
        idl = const.tile([LPC, LPC], F32)
        nc.sync.dma_start(out=idl[:], in_=idl_d[:])
        selm = const.tile([64, C * NCORE], F32)
        nc.sync.dma_start(out=selm[:], in_=selm_d[:])
        xest = const.tile([64, ROUNDS * C], F32)
        nc.sync.dma_start(out=xest[:], in_=xest_d[:])
        mcol = const.tile([64, ROUNDS + 1], F32)
        nc.sync.dma_start(out=mcol[:], in_=mcol_d[:])
        # fcol/fh live on partitions 64:128 so the scalar_tensor_tensor
        # accumulate shares its start partition with lhsT's h half
        fcolw = const.tile([128, ROUNDS], HDT)
        nc.sync.dma_start(out=fcolw[64:128, :], in_=fcol_d[:])
        onesv = const.tile([128, V], F32)
        nc.sync.dma_start(out=onesv[:], in_=ones_d[:])
        vct = const.tile([128, 1], F32)
        nc.sync.dma_start(out=vct[:], in_=vct_d[:])
        wfct = const.tile([64, V], HDT)
        nc.sync.dma_start(out=wfct[:], in_=wfc_d[:])
        bfct = const.tile([V, 1], F32)
        nc.sync.dma_start(out=bfct[:], in_=bfc_d[:])
        iotar = const.tile([128, V], F32)
        nc.sync.dma_start(out=iotar[:], in_=iot_d[:])
        idn = const.tile([V, V], F32)
        nc.sync.dma_start(out=idn[:], in_=idn_d[:])

        # ---- persistent state ----
        warm_act = state.tile([64, 1], F32)
        nc.vector.memset(warm_act[:], 0.0)
        nc.scalar.activation(warm_act[:], warm_act[:], SIG)
        nc.scalar.activation(warm_act[:], warm_act[:],
                             mybir.ActivationFunctionType.Exp)
        lhsT = state.tile([128, LPC], HDT)   # rows 0:64 = y inputs, 64:128 = h
        cst = state.tile([64, LPC], F32)     # cell state
        fhw = state.tile([128, LPC], HDT)    # final-h accumulator (rows 64:128)
        nc.vector.memset(lhsT[:], 0.0)
        nc.vector.memset(cst[:], 0.0)
        nc.vector.memset(fhw[:], 0.0)

        def issue_slot_dma(r):
            """Prefetch agout[r-S] into SBUF (Sync queue, off critical path)."""
            agprev = aginit_d if r < S else agout[r - S]
            slots = work.tile([64, NCORE * C], HDT, tag="slots")
            nc.sync.dma_start(
                out=slots.rearrange("p (s t) -> p s t", t=C),
                in_=agprev[:].rearrange("s p t -> p s t"),
            )
            return slots

        def issue_slot_select(r, slots):
            """inch(r) = sel(slots) + xest[r] (DVE, at round-r start)."""
            tmp8 = work.tile([64, C * NCORE], F32, tag="tmp8")
            nc.vector.tensor_mul(
                tmp8.rearrange("p (t s) -> p t s", s=NCORE),
                slots.rearrange("p (s t) -> p t s", t=C),
                selm.rearrange("p (t s) -> p t s", s=NCORE),
            )
            inch = work.tile([64, C], F32, tag="inch")
            nc.vector.tensor_reduce(
                out=inch[:],
                in_=tmp8.rearrange("p (t s) -> p t s", s=NCORE),
                axis=mybir.AxisListType.X,
                op=ADD_OP,
            )
            nc.vector.tensor_add(inch[:], inch[:], xest[:, r * C:(r + 1) * C])
            return inch

        slots_cur = issue_slot_dma(0)

        for r in range(ROUNDS):
            inch = issue_slot_select(r, slots_cur)
            # state masking for pipeline fill: cst zeroed at round start
            # (h/y were already masked by the previous round's last h write)
            if 0 < r <= FILL_MAX:
                nc.vector.tensor_scalar_mul(cst[:], cst[:], mcol[:, r:r + 1])

            outch = work.tile([64, C], HDT, tag="outch")

            for t in range(C):
                # prefetch next round's slots DMA late in this round
                if t == C - 1 and r + 1 < ROUNDS:
                    slots_next = issue_slot_dma(r + 1)

                # inject this tick's layer-0 input. Mid-round it hides on
                # ACT; at tick 0 it chains off the selects, so do it on DVE
                # to skip a cross-engine handoff.
                if t == 0:
                    nc.vector.tensor_copy(lhsT[0:64, 0:1], inch[:, t:t + 1])
                else:
                    nc.scalar.copy(lhsT[0:64, 0:1], inch[:, t:t + 1])

                gpA = gpool.tile([128, LPC], F32, tag="gA")
                gpB = gpool.tile([128, LPC], F32, tag="gB")
                # seed PSUM with the gate biases, then accumulate the matvecs
                nc.tensor.matmul(gpA[:], bT[:, 0:128], idl[:],
                                 start=True, stop=False, skip_group_check=True)
                nc.tensor.matmul(gpB[:], bT[:, 128:256], idl[:],
                                 start=True, stop=False, skip_group_check=True)
                for l in range(LPC):
                    nc.tensor.matmul(
                        gpA[:, l:l + 1], wsb_v[:, 2 * l, :], lhsT[:, l:l + 1],
                        start=False, stop=True, skip_group_check=True,
                    )
                for l in range(LPC):
                    nc.tensor.matmul(
                        gpB[:, l:l + 1], wsb_v[:, 2 * l + 1, :], lhsT[:, l:l + 1],
                        start=False, stop=True, skip_group_check=True,
                    )

                # gate row order (host-permuted): A = [f|i], B = [o|g]
                sgA = work.tile([128, LPC], F32, tag="sgA")
                sgB = work.tile([128, LPC], F32, tag="sgB")
                nc.scalar.activation(sgA[:], gpA[:], SIG)
                nc.scalar.activation(sgB[64:128, :], gpB[64:128, :], TANH)
                nc.scalar.activation(sgB[0:64, :], gpB[0:64, :], SIG)

                t1 = work.tile([64, LPC], F32, tag="t1")
                nc.vector.tensor_mul(t1[:], sgA[0:64, :], cst[:])
                t2 = work.tile([64, LPC], F32, tag="t2")
                nc.vector.tensor_mul(t2[:], sgA[64:128, :], sgB[64:128, :])
                nc.vector.tensor_add(cst[:], t1[:], t2[:])

                tct = work.tile([64, LPC], F32, tag="tct")
                nc.scalar.activation(tct[:], cst[:], TANH)

                # h = sigmoid(o) * tanh(c); on the round's last tick fold in
                # the next round's fill mask so downstream state starts at 0
                if t == C - 1 and r + 1 <= FILL_MAX:
                    nc.vector.tensor_scalar_mul(tct[:], tct[:], mcol[:, r + 1:r + 2])
                nc.vector.tensor_mul(lhsT[64:128, :], sgB[0:64, :], tct[:])
                nc.vector.tensor_copy(lhsT[0:64, 1:LPC], lhsT[64:128, 0:LPC - 1])
                nc.vector.tensor_copy(outch[:, t:t + 1], lhsT[64:128, LPC - 1:LPC])

            # accumulate the final h on the one round where it's this core's
            if r >= R - 1:
                nc.vector.scalar_tensor_tensor(
                    fhw[64:128, :], lhsT[64:128, :], fcolw[64:128, r:r + 1],
                    fhw[64:128, :], op0=MUL_OP, op1=ADD_OP,
                )

            if r + 1 < ROUNDS:
                slots_cur = slots_next

            # ship this round's boundary chunk (consumed at round r+S)
            if r < NCC:
                nc.scalar.dma_start(out=ccin[r][:], in_=outch[:])
                nc.gpsimd.collective_compute(
                    "AllGather", mybir.AluOpType.bypass, replica_groups=groups,
                    ins=[ccin[r][:]], outs=[agout[r][:]],
                )

        # ---- head: gather final h, logits, softmax over layers, argmax ----
        nc.sync.dma_start(out=hfin[:], in_=fhw[64:128, :])
        nc.gpsimd.collective_compute(
            "AllGather", mybir.AluOpType.bypass, replica_groups=groups,
            ins=[hfin[:]], outs=[hfall[:]],
        )
        HT = state.tile([64, NL], HDT)
        nc.sync.dma_start(
            out=HT.rearrange("p (s i) -> p s i", i=LPC),
            in_=hfall[:].rearrange("s p i -> p s i"),
        )
        logp = gpool.tile([V, NL], F32, tag="logp", bufs=1)
        nc.tensor.matmul(logp[:], wfct[:], HT[:], start=True, stop=True)
        logits = work.tile([V, NL], F32, tag="logits")
        nc.scalar.add(logits[:], logp[:], bfct[:, 0:1])

        mx = work.tile([V, 1], F32, tag="mx")
        nc.vector.tensor_reduce(
            out=mx[:], in_=logits[:], axis=mybir.AxisListType.X, op=mybir.AluOpType.max)
        nmx = work.tile([V, 1], F32, tag="nmx")
        nc.scalar.mul(nmx[:], mx[:], -1.0)
        ex = work.tile([V, NL], F32, tag="ex")
        nc.scalar.activation(
            ex[:], logits[:], mybir.ActivationFunctionType.Exp, bias=nmx[:, 0:1])
        sm = work.tile([V, 1], F32, tag="sm")
        nc.vector.tensor_reduce(
            out=sm[:], in_=ex[:], axis=mybir.AxisListType.X, op=ADD_OP)
        rsm = work.tile([V, 1], F32, tag="rsm")
        nc.vector.reciprocal(rsm[:], sm[:])
        probs = work.tile([V, NL], F32, tag="probs")
        nc.scalar.mul(probs[:], ex[:], rsm[:, 0:1])

        tp = gpool.tile([128, V], F32, tag="tp", bufs=1)
        nc.tensor.transpose(tp[:], probs[:], idn[:])
        m2 = work.tile([128, 1], F32, tag="m2")
        nc.vector.tensor_reduce(
            out=m2[:], in_=tp[:], axis=mybir.AxisListType.X, op=mybir.AluOpType.max)
        m2b = work.tile([128, V], F32, tag="m2b")
        nc.scalar.mul(m2b[:], onesv[:], m2[:, 0:1])
        eq = work.tile([128, V], F32, tag="eq")
        nc.vector.tensor_tensor(eq[:], tp[:], m2b[:], op=mybir.AluOpType.is_equal)
        val = work.tile([128, V], F32, tag="val")
        nc.vector.tensor_mul(val[:], eq[:], iotar[:])
        mr = work.tile([128, 1], F32, tag="mr")
        nc.vector.tensor_reduce(
            out=mr[:], in_=val[:], axis=mybir.AxisListType.X, op=mybir.AluOpType.max)
        idx = work.tile([128, 1], F32, tag="idx")
        nc.vector.tensor_sub(idx[:], vct[:], mr[:])
        nc.scalar.dma_start(out=out_d[:], in_=idx[:])

    nc.finalize()
    return nc


def _prep_in_maps(inputs):
    x = np.asarray(inputs["x"]).astype(np.int64)
    embed = np.asarray(inputs["embed"], dtype=np.float32)
    xe = embed[x, 0]  # (T,)

    Wih_full = np.zeros((NL, 4 * H, H), np.float32)
    Wih_full[0, :, 0] = np.asarray(inputs["Wih0"], np.float32)[:, 0]
    Wih_full[1:] = np.asarray(inputs["Wih"], np.float32)
    Whh_full = np.concatenate(
        [np.asarray(inputs["Whh0"], np.float32)[None],
         np.asarray(inputs["Whh"], np.float32)], axis=0)
    b_full = np.concatenate(
        [(np.asarray(inputs["bih0"], np.float32)
          + np.asarray(inputs["bhh0"], np.float32))[None],
         np.asarray(inputs["bih"], np.float32)
         + np.asarray(inputs["bhh"], np.float32)], axis=0)  # (NL, 256)

    Wcat = np.concatenate([Wih_full, Whh_full], axis=2)      # (NL, 256, 128)
    # permute pytorch gate order [i f g o] -> [f i o g] so the device layout
    # has half-A rows = [f; i] and half-B rows = [o; g]
    perm = np.r_[64:128, 0:64, 192:256, 128:192]
    Wcat = Wcat[:, perm, :]
    b_full = b_full[:, perm]
    lhsT_all = np.ascontiguousarray(np.transpose(Wcat, (0, 2, 1)))  # (NL,128,256)

    wfct = np.asarray(inputs["Wfc"], np.float32).T.astype(HDT_NP)  # (64, V)
    bfc = np.asarray(inputs["bfc"], np.float32).reshape(V, 1)
    iotar = np.broadcast_to(
        (V - np.arange(V, dtype=np.float32))[None, :], (128, V)).copy()
    idn = np.eye(V, dtype=np.float32)
    aginit = np.zeros((NCORE, 64, C), HDT_NP)

    in_maps = []
    for k in range(NCORE):
        lhsT_k = lhsT_all[k * LPC:(k + 1) * LPC]  # (LPC, 128, 256)
        wts = (lhsT_k.reshape(LPC, 128, 2, 128)
               .transpose(0, 2, 1, 3)
               .reshape(2 * LPC, 128, 128).astype(HDT_NP))
        wts = np.ascontiguousarray(wts.transpose(1, 0, 2)).reshape(128, 2 * LPC * 128)
        bT = np.ascontiguousarray(b_full[k * LPC:(k + 1) * LPC].astype(np.float32))  # (LPC, 256)

        selm = np.zeros((64, C, NCORE), np.float32)
        if k > 0:
            selm[:, :, k - 1] = 1.0
        xest = np.zeros((64, ROUNDS * C), np.float32)
        if k == 0:
            xest[0, :T] = xe
        mcol = np.ones((64, ROUNDS + 1), np.float32)
        mcol[:, :S * k + 1] = 0.0  # zero state at start of rounds <= S*k
        fcol = np.zeros((64, ROUNDS), HDT_NP)
        fcol[:, S * k + R - 1] = 1.0

        in_maps.append({
            "wts": wts,
            "bT": bT,
            "idl": np.eye(LPC, dtype=np.float32),
            "selm": selm.reshape(64, C * NCORE),
            "xest": xest,
            "mcol": mcol,
            "fcol": fcol,
            "onesv": np.ones((128, V), np.float32),
            "vct": np.full((128, 1), float(V), np.float32),
            "wfct": wfct,
            "bfc": bfc,
            "iotar": iotar,
            "idn": idn,
            "aginit": aginit,
        })
    return in_maps


def _run(inputs, trace=False):
    if "nc" not in _CACHE:
        _CACHE["nc"] = _build()
    nc = _CACHE["nc"]
    in_maps = _prep_in_maps(inputs)
    res = run_bass_kernel_spmd(nc, in_maps, list(range(NCORE)), trace=trace)
    out = np.asarray(res.results[0]["out_idx"], np.float32).reshape(NL)
    idx = np.rint(out).astype(np.int32)
    return idx, res


def kernel(**inputs) -> np.ndarray:
    idx, _ = _run(inputs, trace=False)
    return idx


# revision 32
# speedup vs baseline: 1.5300x; 1.0607x over previous
"""CharRNN (128-layer stacked LSTM, H=64, T=128, B=1) on 8 Trainium2 cores.

Strategy: pipeline-parallel over layers (16 layers/core), wavefront ticks
inside each core. Per LSTM cell the two gate-half matvecs are computed as
  matmul(out=(128,1) psum column, lhsT=W_half^T (128x128, stationary, fp16),
         rhs=[y; h] (128,1))
so the 16 cells of a tick form (128,16) gate tiles with gates on partitions
and cells on the free dim. Optimizations over the v0 kernel:
  - bias folded into PSUM via a seed matmul (lhsT=bias rows, rhs=identity)
    so activations read gates straight from PSUM (no bias add on DVE);
  - gate halves A=[f;i] / B=[o;g] are separate PSUM tiles and bursts, so
    sigmoid(A) overlaps the B matmul burst;
  - pipeline skew S: round r consumes the AllGather issued at round r-S,
    hiding collective latency + slot-select behind a full round of compute;
  - engine split: ACT does inject+activations, DVE does products/copies,
    GPSIMD only slot-selects + collectives; per-round work is hoisted a
    round early so round boundaries cost nothing on the critical path;
  - state zeroing for pipeline fill is folded into the last-tick h write
    (scalar_tensor_tensor with a per-core mask column).
"""

import sys

sys.path.insert(0, "/opt/trn_rl_repo")

from contextlib import ExitStack

import numpy as np

import concourse.bass as bass
import concourse.mybir as mybir
from concourse import bacc, tile
from concourse.bass_utils import run_bass_kernel_spmd

F32 = mybir.dt.float32
HDT = mybir.dt.float16
HDT_NP = np.float16

H = 64
NL = 128
T = 128
V = 35
NCORE = 8
LPC = NL // NCORE          # 16 layers per core
C = 4                      # timesteps per pipeline round
S = 2                      # skew: round r consumes the AllGather of round r-S
R = T // C
ROUNDS = R + S * (NCORE - 1)
FILL_MAX = S * (NCORE - 1)  # rounds <= S*k have core k's state zeroed

_CACHE = {}

SIG = mybir.ActivationFunctionType.Sigmoid
TANH = mybir.ActivationFunctionType.Tanh
MUL_OP = mybir.AluOpType.mult
ADD_OP = mybir.AluOpType.add


def _build():
    nc = bacc.Bacc()

    wts_d = nc.declare_dram_parameter("wts", [128, 2 * LPC * 128], HDT, isOutput=False)
    bT_d = nc.declare_dram_parameter("bT", [LPC, 2 * 128], F32, isOutput=False)
    idl_d = nc.declare_dram_parameter("idl", [LPC, LPC], F32, isOutput=False)
    selm_d = nc.declare_dram_parameter("selm", [64, C * NCORE], F32, isOutput=False)
    xest_d = nc.declare_dram_parameter("xest", [64, ROUNDS * C], F32, isOutput=False)
    mcol_d = nc.declare_dram_parameter("mcol", [64, ROUNDS + 1], F32, isOutput=False)
    fcol_d = nc.declare_dram_parameter("fcol", [64, ROUNDS], HDT, isOutput=False)
    ones_d = nc.declare_dram_parameter("onesv", [128, V], F32, isOutput=False)
    vct_d = nc.declare_dram_parameter("vct", [128, 1], F32, isOutput=False)
    wfc_d = nc.declare_dram_parameter("wfct", [64, V], HDT, isOutput=False)
    bfc_d = nc.declare_dram_parameter("bfc", [V, 1], F32, isOutput=False)
    iot_d = nc.declare_dram_parameter("iotar", [128, V], F32, isOutput=False)
    idn_d = nc.declare_dram_parameter("idn", [V, V], F32, isOutput=False)
    aginit_d = nc.declare_dram_parameter("aginit", [NCORE, 64, C], HDT, isOutput=False)
    out_d = nc.declare_dram_parameter("out_idx", [128, 1], F32, isOutput=True)

    NCC = ROUNDS - S  # collectives actually consumed
    ccin = [nc.dram_tensor(f"ccin{r}", [64, C], HDT) for r in range(NCC)]
    agout = [
        nc.dram_tensor(f"agout{r}", [NCORE, 64, C], HDT, addr_space="Shared")
        for r in range(NCC)
    ]
    hfin = nc.dram_tensor("hfin", [64, LPC], HDT)
    hfall = nc.dram_tensor("hfall", [NCORE, 64, LPC], HDT, addr_space="Shared")
    warm_in = nc.dram_tensor("warm_in", [64, C], HDT)
    warm_out = nc.dram_tensor("warm_out", [NCORE, 64, C], HDT, addr_space="Shared")

    groups = [list(range(NCORE))]

    with tile.TileContext(nc) as tc, ExitStack() as ctx:
        # warmup AllGather: starts collective-engine init at t=0 (it is
        # otherwise lazy, ~90us) and syncs the cores' CC pipelines before
        # round 0's collective lands on them
        const = ctx.enter_context(tc.tile_pool(name="const", bufs=1))
        # no input dependency: garbage data is fine, nothing consumes it
        nc.gpsimd.collective_compute(
            "AllGather", mybir.AluOpType.bypass, replica_groups=[list(range(NCORE))],
            ins=[warm_in[:]], outs=[warm_out[:]],
        )
        state = ctx.enter_context(tc.tile_pool(name="state", bufs=1))
        work = ctx.enter_context(tc.tile_pool(name="work", bufs=3))
        gpool = ctx.enter_context(tc.tile_pool(name="gpool", bufs=2, space="PSUM"))

        # ---- constants ----
        wsb = const.tile([128, 2 * LPC * 128], HDT)
        nc.sync.dma_start(out=wsb[:], in_=wts_d[:])
        wsb_v = wsb.rearrange("k (j m) -> k j m", m=128)
        bT = const.tile([LPC, 2 * 128], F32)
        nc.sync.dma_start(out=bT[:], in_=bT_d[:])
        idl = const.tile([LPC, LPC], F32)
        nc.sync.dma_start(out=idl[:], in_=idl_d[:])
        selm = const.tile([64, C * NCORE], F32)
        nc.sync.dma_start(out=selm[:], in_=selm_d[:])
        xest = const.tile([64, ROUNDS * C], F32)
        nc.sync.dma_start(out=xest[:], in_=xest_d[:])
        mcol = const.tile([64, ROUNDS + 1], F32)
        nc.sync.dma_start(out=mcol[:], in_=mcol_d[:])
        # fcol/fh live on partitions 64:128 so the scalar_tensor_tensor
        # accumulate shares its start partition with lhsT's h half
        fcolw = const.tile([128, ROUNDS], HDT)
        nc.sync.dma_start(out=fcolw[64:128, :], in_=fcol_d[:])
        onesv = const.tile([128, V], F32)
        nc.sync.dma_start(out=onesv[:], in_=ones_d[:])
        vct = const.tile([128, 1], F32)
        nc.sync.dma_start(out=vct[:], in_=vct_d[:])
        wfct = const.tile([64, V], HDT)
        nc.sync.dma_start(out=wfct[:], in_=wfc_d[:])
        bfct = const.tile([V, 1], F32)
        nc.sync.dma_start(out=bfct[:], in_=bfc_d[:])
        iotar = const.tile([128, V], F32)
        nc.sync.dma_start(out=iotar[:], in_=iot_d[:])
        idn = const.tile([V, V], F32)
        nc.sync.dma_start(out=idn[:], in_=idn_d[:])

        # ---- persistent state ----
        warm_act = state.tile([64, 1], F32)
        nc.vector.memset(warm_act[:], 0.0)
        nc.scalar.activation(warm_act[:], warm_act[:], SIG)
        nc.scalar.activation(warm_act[:], warm_act[:],
                             mybir.ActivationFunctionType.Exp)
        lhsT = state.tile([128, LPC], HDT)   # rows 0:64 = y inputs, 64:128 = h
        cst = state.tile([64, LPC], F32)     # cell state
        fhw = state.tile([128, LPC], HDT)    # final-h accumulator (rows 64:128)
        nc.vector.memset(lhsT[:], 0.0)
        nc.vector.memset(cst[:], 0.0)
        nc.vector.memset(fhw[:], 0.0)

        def issue_slot_dma(r):
            """Prefetch agout[r-S] into SBUF (Sync queue, off critical path)."""
            agprev = aginit_d if r < S else agout[r - S]
            slots = work.tile([64, NCORE * C], HDT, tag="slots")
            nc.sync.dma_start(
                out=slots.rearrange("p (s t) -> p s t", t=C),
                in_=agprev[:].rearrange("s p t -> p s t"),
            )
            return slots

        def issue_slot_select(r, slots):
            """inch(r) = sel(slots) + xest[r] (DVE, at round-r start)."""
            tmp8 = work.tile([64, C * NCORE], F32, tag="tmp8")
            nc.vector.tensor_mul(
                tmp8.rearrange("p (t s) -> p t s", s=NCORE),
                slots.rearrange("p (s t) -> p t s", t=C),
                selm.rearrange("p (t s) -> p t s", s=NCORE),
            )
            inch = work.tile([64, C], F32, tag="inch")
            nc.vector.tensor_reduce(
                out=inch[:],
                in_=tmp8.rearrange("p (t s) -> p t s", s=NCORE),
                axis=mybir.AxisListType.X,
                op=ADD_OP,
            )
            nc.vector.tensor_add(inch[:], inch[:], xest[:, r * C:(r + 1) * C])
            return inch

        slots_cur = issue_slot_dma(0)

        for r in range(ROUNDS):
            inch = issue_slot_select(r, slots_cur)
            # state masking for pipeline fill: cst zeroed at round start
            # (h/y were already masked by the previous round's last h write)
            if 0 < r <= FILL_MAX:
                nc.vector.tensor_scalar_mul(cst[:], cst[:], mcol[:, r:r + 1])

            outch = work.tile([64, C], HDT, tag="outch")

            for t in range(C):
                # prefetch next round's slots DMA late in this round
                if t == C - 1 and r + 1 < ROUNDS:
                    slots_next = issue_slot_dma(r + 1)

                # inject this tick's layer-0 input. Mid-round it hides on
                # ACT; at tick 0 it chains off the selects, so do it on DVE
                # to skip a cross-engine handoff.
                if t == 0:
                    nc.vector.tensor_copy(lhsT[0:64, 0:1], inch[:, t:t + 1])
                else:
                    nc.scalar.copy(lhsT[0:64, 0:1], inch[:, t:t + 1])

                gpA = gpool.tile([128, LPC], F32, tag="gA")
                gpB = gpool.tile([128, LPC], F32, tag="gB")
                # seed PSUM with the gate biases, then accumulate the matvecs
                nc.tensor.matmul(gpA[:], bT[:, 0:128], idl[:],
                                 start=True, stop=False, skip_group_check=True)
                nc.tensor.matmul(gpB[:], bT[:, 128:256], idl[:],
                                 start=True, stop=False, skip_group_check=True)
                for l in range(LPC):
                    nc.tensor.matmul(
                        gpA[:, l:l + 1], wsb_v[:, 2 * l, :], lhsT[:, l:l + 1],
                        start=False, stop=True, skip_group_check=True,
                    )
                for l in range(LPC):
                    nc.tensor.matmul(
                        gpB[:, l:l + 1], wsb_v[:, 2 * l + 1, :], lhsT[:, l:l + 1],
                        start=False, stop=True, skip_group_check=True,
                    )

                # gate row order (host-permuted): A = [f|i], B = [o|g]
                sgA = work.tile([128, LPC], F32, tag="sgA")
                sgB = work.tile([128, LPC], F32, tag="sgB")
                nc.scalar.activation(sgA[:], gpA[:], SIG)
                nc.scalar.activation(sgB[64:128, :], gpB[64:128, :], TANH)
                nc.scalar.activation(sgB[0:64, :], gpB[0:64, :], SIG)

                t1 = work.tile([64, LPC], F32, tag="t1")
                nc.vector.tensor_mul(t1[:], sgA[0:64, :], cst[:])
                t2 = work.tile([64, LPC], F32, tag="t2")
                nc.vector.tensor_mul(t2[:], sgA[64:128, :], sgB[64:128, :])
                nc.vector.tensor_add(cst[:], t1[:], t2[:])

                tct = work.tile([64, LPC], F32, tag="tct")
                nc.scalar.activation(tct[:], cst[:], TANH)

                # h = sigmoid(o) * tanh(c); on the round's last tick fold in
                # the next round's fill mask so downstream state starts at 0
                if t == C - 1 and r + 1 <= FILL_MAX:
                    nc.vector.tensor_scalar_mul(tct[:], tct[:], mcol[:, r + 1:r + 2])
                # write the shifted y half directly (same product as h), then
                # the h half: the next burst then has a single gate (hmul)
                # instead of hmul->shift, avoiding a mid-burst PE restart
                nc.vector.tensor_mul(lhsT[0:64, 1:LPC], sgB[0:64, 0:LPC - 1],
                                     tct[:, 0:LPC - 1])
                nc.vector.tensor_mul(lhsT[64:128, :], sgB[0:64, :], tct[:])
                nc.vector.tensor_copy(outch[:, t:t + 1], lhsT[64:128, LPC - 1:LPC])

            # accumulate the final h on the one round where it's this core's
            if r >= R - 1:
                nc.vector.scalar_tensor_tensor(
                    fhw[64:128, :], lhsT[64:128, :], fcolw[64:128, r:r + 1],
                    fhw[64:128, :], op0=MUL_OP, op1=ADD_OP,
                )

            if r + 1 < ROUNDS:
                slots_cur = slots_next

            # ship this round's boundary chunk (consumed at round r+S)
            if r < NCC:
                nc.scalar.dma_start(out=ccin[r][:], in_=outch[:])
                nc.gpsimd.collective_compute(
                    "AllGather", mybir.AluOpType.bypass, replica_groups=groups,
                    ins=[ccin[r][:]], outs=[agout[r][:]],
                )

        # ---- head: gather final h, logits, softmax over layers, argmax ----
        nc.sync.dma_start(out=hfin[:], in_=fhw[64:128, :])
        nc.gpsimd.collective_compute(
            "AllGather", mybir.AluOpType.bypass, replica_groups=groups,
            ins=[hfin[:]], outs=[hfall[:]],
        )
        HT = state.tile([64, NL], HDT)
        nc.sync.dma_start(
            out=HT.rearrange("p (s i) -> p s i", i=LPC),
            in_=hfall[:].rearrange("s p i -> p s i"),
        )
        logp = gpool.tile([V, NL], F32, tag="logp", bufs=1)
        nc.tensor.matmul(logp[:], wfct[:], HT[:], start=True, stop=True)
        logits = work.tile([V, NL], F32, tag="logits")
        nc.scalar.add(logits[:], logp[:], bfct[:, 0:1])

        mx = work.tile([V, 1], F32, tag="mx")
        nc.vector.tensor_reduce(
            out=mx[:], in_=logits[:], axis=mybir.AxisListType.X, op=mybir.AluOpType.max)
        nmx = work.tile([V, 1], F32, tag="nmx")
        nc.scalar.mul(nmx[:], mx[:], -1.0)
        ex = work.tile([V, NL], F32, tag="ex")
        nc.scalar.activation(
            ex[:], logits[:], mybir.ActivationFunctionType.Exp, bias=nmx[:, 0:1])
        sm = work.tile([V, 1], F32, tag="sm")
        nc.vector.tensor_reduce(
            out=sm[:], in_=ex[:], axis=mybir.AxisListType.X, op=ADD_OP)
        rsm = work.tile([V, 1], F32, tag="rsm")
        nc.vector.reciprocal(rsm[:], sm[:])
        probs = work.tile([V, NL], F32, tag="probs")
        nc.scalar.mul(probs[:], ex[:], rsm[:, 0:1])

        tp = gpool.tile([128, V], F32, tag="tp", bufs=1)
        nc.tensor.transpose(tp[:], probs[:], idn[:])
        m2 = work.tile([128, 1], F32, tag="m2")
        nc.vector.tensor_reduce(
            out=m2[:], in_=tp[:], axis=mybir.AxisListType.X, op=mybir.AluOpType.max)
        m2b = work.tile([128, V], F32, tag="m2b")
        nc.scalar.mul(m2b[:], onesv[:], m2[:, 0:1])
        eq = work.tile([128, V], F32, tag="eq")
        nc.vector.tensor_tensor(eq[:], tp[:], m2b[:], op=mybir.AluOpType.is_equal)
        val = work.tile([128, V], F32, tag="val")
        nc.vector.tensor_mul(val[:], eq[:], iotar[:])
        mr = work.tile([128, 1], F32, tag="mr")
        nc.vector.tensor_reduce(
            out=mr[:], in_=val[:], axis=mybir.AxisListType.X, op=mybir.AluOpType.max)
        idx = work.tile([128, 1], F32, tag="idx")
        nc.vector.tensor_sub(idx[:], vct[:], mr[:])
        nc.scalar.dma_start(out=out_d[:], in_=idx[:])

    nc.finalize()
    return nc


def _prep_in_maps(inputs):
    x = np.asarray(inputs["x"]).astype(np.int64)
    embed = np.asarray(inputs["embed"], dtype=np.float32)
    xe = embed[x, 0]  # (T,)

    Wih_full = np.zeros((NL, 4 * H, H), np.float32)
    Wih_full[0, :, 0] = np.asarray(inputs["Wih0"], np.float32)[:, 0]
    Wih_full[1:] = np.asarray(inputs["Wih"], np.float32)
    Whh_full = np.concatenate(
        [np.asarray(inputs["Whh0"], np.float32)[None],
         np.asarray(inputs["Whh"], np.float32)], axis=0)
    b_full = np.concatenate(
        [(np.asarray(inputs["bih0"], np.float32)
          + np.asarray(inputs["bhh0"], np.float32))[None],
         np.asarray(inputs["bih"], np.float32)
         + np.asarray(inputs["bhh"], np.float32)], axis=0)  # (NL, 256)

    Wcat = np.concatenate([Wih_full, Whh_full], axis=2)      # (NL, 256, 128)
    # permute pytorch gate order [i f g o] -> [f i o g] so the device layout
    # has half-A rows = [f; i] and half-B rows = [o; g]
    perm = np.r_[64:128, 0:64, 192:256, 128:192]
    Wcat = Wcat[:, perm, :]
    b_full = b_full[:, perm]
    lhsT_all = np.ascontiguousarray(np.transpose(Wcat, (0, 2, 1)))  # (NL,128,256)

    wfct = np.asarray(inputs["Wfc"], np.float32).T.astype(HDT_NP)  # (64, V)
    bfc = np.asarray(inputs["bfc"], np.float32).reshape(V, 1)
    iotar = np.broadcast_to(
        (V - np.arange(V, dtype=np.float32))[None, :], (128, V)).copy()
    idn = np.eye(V, dtype=np.float32)
    aginit = np.zeros((NCORE, 64, C), HDT_NP)

    in_maps = []
    for k in range(NCORE):
        lhsT_k = lhsT_all[k * LPC:(k + 1) * LPC]  # (LPC, 128, 256)
        wts = (lhsT_k.reshape(LPC, 128, 2, 128)
               .transpose(0, 2, 1, 3)
               .reshape(2 * LPC, 128, 128).astype(HDT_NP))
        wts = np.ascontiguousarray(wts.transpose(1, 0, 2)).reshape(128, 2 * LPC * 128)
        bT = np.ascontiguousarray(b_full[k * LPC:(k + 1) * LPC].astype(np.float32))  # (LPC, 256)

        selm = np.zeros((64, C, NCORE), np.float32)
        if k > 0:
            selm[:, :, k - 1] = 1.0
        xest = np.zeros((64, ROUNDS * C), np.float32)
        if k == 0:
            xest[0, :T] = xe
        mcol = np.ones((64, ROUNDS + 1), np.float32)
        mcol[:, :S * k + 1] = 0.0  # zero state at start of rounds <= S*k
        fcol = np.zeros((64, ROUNDS), HDT_NP)
        fcol[:, S * k + R - 1] = 1.0

        in_maps.append({
            "wts": wts,
            "bT": bT,
            "idl": np.eye(LPC, dtype=np.float32),
            "selm": selm.reshape(64, C * NCORE),
            "xest": xest,
            "mcol": mcol,
            "fcol": fcol,
            "onesv": np.ones((128, V), np.float32),
            "vct": np.full((128, 1), float(V), np.float32),
            "wfct": wfct,
            "bfc": bfc,
            "iotar": iotar,
            "idn": idn,
            "aginit": aginit,
        })
    return in_maps


def _run(inputs, trace=False):
    if "nc" not in _CACHE:
        _CACHE["nc"] = _build()
    nc = _CACHE["nc"]
    in_maps = _prep_in_maps(inputs)
    res = run_bass_kernel_spmd(nc, in_maps, list(range(NCORE)), trace=trace)
    out = np.asarray(res.results[0]["out_idx"], np.float32).reshape(NL)
    idx = np.rint(out).astype(np.int32)
    return idx, res


def kernel(**inputs) -> np.ndarray:
    idx, _ = _run(inputs, trace=False)
    return idx


# revision 33
# speedup vs baseline: 1.5898x; 1.0391x over previous
"""CharRNN (128-layer stacked LSTM, H=64, T=128, B=1) on 8 Trainium2 cores.

Strategy: pipeline-parallel over layers (16 layers/core), wavefront ticks
inside each core. Per LSTM cell the two gate-half matvecs are computed as
  matmul(out=(128,1) psum column, lhsT=W_half^T (128x128, stationary, fp16),
         rhs=[y; h] (128,1))
so the 16 cells of a tick form (128,16) gate tiles with gates on partitions
and cells on the free dim. Optimizations over the v0 kernel:
  - bias folded into PSUM via a seed matmul (lhsT=bias rows, rhs=identity)
    so activations read gates straight from PSUM (no bias add on DVE);
  - gate halves A=[f;i] / B=[o;g] are separate PSUM tiles and bursts, so
    sigmoid(A) overlaps the B matmul burst;
  - pipeline skew S: round r consumes the AllGather issued at round r-S,
    hiding collective latency + slot-select behind a full round of compute;
  - engine split: ACT does inject+activations, DVE does products/copies,
    GPSIMD only slot-selects + collectives; per-round work is hoisted a
    round early so round boundaries cost nothing on the critical path;
  - state zeroing for pipeline fill is folded into the last-tick h write
    (scalar_tensor_tensor with a per-core mask column).
"""

import sys

sys.path.insert(0, "/opt/trn_rl_repo")

from contextlib import ExitStack

import numpy as np

import concourse.bass as bass
import concourse.mybir as mybir
from concourse import bacc, tile
from concourse.bass_utils import run_bass_kernel_spmd

F32 = mybir.dt.float32
HDT = mybir.dt.float16
HDT_NP = np.float16

H = 64
NL = 128
T = 128
V = 35
NCORE = 8
LPC = NL // NCORE          # 16 layers per core
C = 4                      # timesteps per pipeline round
S = 2                      # skew: round r consumes the AllGather of round r-S
R = T // C
ROUNDS = R + S * (NCORE - 1)
FILL_MAX = S * (NCORE - 1)  # rounds <= S*k have core k's state zeroed

_CACHE = {}

SIG = mybir.ActivationFunctionType.Sigmoid
TANH = mybir.ActivationFunctionType.Tanh
MUL_OP = mybir.AluOpType.mult
ADD_OP = mybir.AluOpType.add


def _build():
    nc = bacc.Bacc()

    wts_d = nc.declare_dram_parameter("wts", [128, 2 * LPC * 128], HDT, isOutput=False)
    bT_d = nc.declare_dram_parameter("bT", [LPC, 2 * 128], F32, isOutput=False)
    idl_d = nc.declare_dram_parameter("idl", [LPC, LPC], F32, isOutput=False)
    selm_d = nc.declare_dram_parameter("selm", [64, C * NCORE], F32, isOutput=False)
    xest_d = nc.declare_dram_parameter("xest", [64, ROUNDS * C], F32, isOutput=False)
    mcol_d = nc.declare_dram_parameter("mcol", [64, ROUNDS + 1], F32, isOutput=False)
    fcol_d = nc.declare_dram_parameter("fcol", [64, ROUNDS], HDT, isOutput=False)
    ones_d = nc.declare_dram_parameter("onesv", [128, V], F32, isOutput=False)
    vct_d = nc.declare_dram_parameter("vct", [128, 1], F32, isOutput=False)
    wfc_d = nc.declare_dram_parameter("wfct", [64, V], HDT, isOutput=False)
    bfc_d = nc.declare_dram_parameter("bfc", [V, 1], F32, isOutput=False)
    iot_d = nc.declare_dram_parameter("iotar", [128, V], F32, isOutput=False)
    idn_d = nc.declare_dram_parameter("idn", [V, V], F32, isOutput=False)
    aginit_d = nc.declare_dram_parameter("aginit", [NCORE, 64, C], HDT, isOutput=False)
    out_d = nc.declare_dram_parameter("out_idx", [128, 1], F32, isOutput=True)

    NCC = ROUNDS - S  # collectives actually consumed
    ccin = [nc.dram_tensor(f"ccin{r}", [64, C], HDT) for r in range(NCC)]
    agout = [
        nc.dram_tensor(f"agout{r}", [NCORE, 64, C], HDT, addr_space="Shared")
        for r in range(NCC)
    ]
    hfin = nc.dram_tensor("hfin", [64, LPC], HDT)
    hfall = nc.dram_tensor("hfall", [NCORE, 64, LPC], HDT, addr_space="Shared")
    warm_in = nc.dram_tensor("warm_in", [64, C], HDT)
    warm_out = nc.dram_tensor("warm_out", [NCORE, 64, C], HDT, addr_space="Shared")

    groups = [list(range(NCORE))]

    with tile.TileContext(nc) as tc, ExitStack() as ctx:
        # warmup AllGather: starts collective-engine init at t=0 (it is
        # otherwise lazy, ~90us) and syncs the cores' CC pipelines before
        # round 0's collective lands on them
        const = ctx.enter_context(tc.tile_pool(name="const", bufs=1))
        # no input dependency: garbage data is fine, nothing consumes it
        nc.gpsimd.collective_compute(
            "AllGather", mybir.AluOpType.bypass, replica_groups=[list(range(NCORE))],
            ins=[warm_in[:]], outs=[warm_out[:]],
        )
        state = ctx.enter_context(tc.tile_pool(name="state", bufs=1))
        work = ctx.enter_context(tc.tile_pool(name="work", bufs=3))
        gpool = ctx.enter_context(tc.tile_pool(name="gpool", bufs=2, space="PSUM"))

        # ---- constants ----
        wsb = const.tile([128, 2 * LPC * 128], HDT)
        nc.sync.dma_start(out=wsb[:], in_=wts_d[:])
        wsb_v = wsb.rearrange("k (j m) -> k j m", m=128)
        bT = const.tile([LPC, 2 * 128], F32)
        nc.sync.dma_start(out=bT[:], in_=bT_d[:])
        idl = const.tile([LPC, LPC], F32)
        nc.sync.dma_start(out=idl[:], in_=idl_d[:])
        selm = const.tile([64, C * NCORE], F32)
        nc.sync.dma_start(out=selm[:], in_=selm_d[:])
        xest = const.tile([64, ROUNDS * C], F32)
        nc.sync.dma_start(out=xest[:], in_=xest_d[:])
        mcol = const.tile([64, ROUNDS + 1], F32)
        nc.sync.dma_start(out=mcol[:], in_=mcol_d[:])
        # fcol/fh live on partitions 64:128 so the scalar_tensor_tensor
        # accumulate shares its start partition with lhsT's h half
        fcolw = const.tile([128, ROUNDS], HDT)
        nc.sync.dma_start(out=fcolw[64:128, :], in_=fcol_d[:])
        onesv = const.tile([128, V], F32)
        nc.sync.dma_start(out=onesv[:], in_=ones_d[:])
        vct = const.tile([128, 1], F32)
        nc.sync.dma_start(out=vct[:], in_=vct_d[:])
        wfct = const.tile([64, V], HDT)
        nc.sync.dma_start(out=wfct[:], in_=wfc_d[:])
        bfct = const.tile([V, 1], F32)
        nc.sync.dma_start(out=bfct[:], in_=bfc_d[:])
        iotar = const.tile([128, V], F32)
        nc.sync.dma_start(out=iotar[:], in_=iot_d[:])
        idn = const.tile([V, V], F32)
        nc.sync.dma_start(out=idn[:], in_=idn_d[:])

        # ---- persistent state ----
        warm_act = state.tile([64, 1], F32)
        nc.vector.memset(warm_act[:], 0.0)
        nc.scalar.activation(warm_act[:], warm_act[:], SIG)
        nc.scalar.activation(warm_act[:], warm_act[:],
                             mybir.ActivationFunctionType.Exp)
        lhsT = state.tile([128, LPC], HDT)   # rows 0:64 = y inputs, 64:128 = h
        cst = state.tile([64, LPC], F32)     # cell state
        fhw = state.tile([128, LPC], HDT)    # final-h accumulator (rows 64:128)
        nc.vector.memset(lhsT[:], 0.0)
        nc.vector.memset(cst[:], 0.0)
        nc.vector.memset(fhw[:], 0.0)

        def issue_slot_dma(r):
            """Prefetch agout[r-S] into SBUF (Sync queue, off critical path)."""
            agprev = aginit_d if r < S else agout[r - S]
            slots = work.tile([64, NCORE * C], HDT, tag="slots")
            nc.sync.dma_start(
                out=slots.rearrange("p (s t) -> p s t", t=C),
                in_=agprev[:].rearrange("s p t -> p s t"),
            )
            return slots

        def issue_slot_select(r, slots):
            """inch(r) = sel(slots) + xest[r] (DVE, at round-r start)."""
            tmp8 = work.tile([64, C * NCORE], F32, tag="tmp8")
            nc.vector.tensor_mul(
                tmp8.rearrange("p (t s) -> p t s", s=NCORE),
                slots.rearrange("p (s t) -> p t s", t=C),
                selm.rearrange("p (t s) -> p t s", s=NCORE),
            )
            inch = work.tile([64, C], F32, tag="inch")
            nc.vector.tensor_reduce(
                out=inch[:],
                in_=tmp8.rearrange("p (t s) -> p t s", s=NCORE),
                axis=mybir.AxisListType.X,
                op=ADD_OP,
            )
            nc.vector.tensor_add(inch[:], inch[:], xest[:, r * C:(r + 1) * C])
            return inch

        slots_cur = issue_slot_dma(0)

        for r in range(ROUNDS):
            inch = issue_slot_select(r, slots_cur)
            # state masking for pipeline fill: cst zeroed at round start
            # (h/y were already masked by the previous round's last h write)
            if 0 < r <= FILL_MAX:
                nc.vector.tensor_scalar_mul(cst[:], cst[:], mcol[:, r:r + 1])

            outch = work.tile([64, C], HDT, tag="outch")

            for t in range(C):
                # prefetch next round's slots DMA late in this round
                if t == C - 1 and r + 1 < ROUNDS:
                    slots_next = issue_slot_dma(r + 1)

                # inject this tick's layer-0 input. Mid-round it hides on
                # ACT; at tick 0 it chains off the selects, so do it on DVE
                # to skip a cross-engine handoff.
                if t == 0:
                    nc.vector.tensor_copy(lhsT[0:64, 0:1], inch[:, t:t + 1])
                else:
                    nc.scalar.copy(lhsT[0:64, 0:1], inch[:, t:t + 1])

                gpA = gpool.tile([128, LPC], F32, tag="gA")
                gpB = gpool.tile([128, LPC], F32, tag="gB")
                # seed PSUM with the gate biases, then accumulate the matvecs
                nc.tensor.matmul(gpA[:], bT[:, 0:128], idl[:],
                                 start=True, stop=False, skip_group_check=True)
                nc.tensor.matmul(gpB[:], bT[:, 128:256], idl[:],
                                 start=True, stop=False, skip_group_check=True)
                # layer 0 last: its column is gated on the inject (and at
                # round boundaries on the slot-selects), so the other 15
                # columns keep the PE busy while it lands
                col_order = list(range(1, LPC)) + [0]
                for l in col_order:
                    nc.tensor.matmul(
                        gpA[:, l:l + 1], wsb_v[:, 2 * l, :], lhsT[:, l:l + 1],
                        start=False, stop=True, skip_group_check=True,
                    )
                for l in col_order:
                    nc.tensor.matmul(
                        gpB[:, l:l + 1], wsb_v[:, 2 * l + 1, :], lhsT[:, l:l + 1],
                        start=False, stop=True, skip_group_check=True,
                    )

                # gate row order (host-permuted): A = [f|i], B = [o|g]
                sgA = work.tile([128, LPC], F32, tag="sgA")
                sgB = work.tile([128, LPC], F32, tag="sgB")
                nc.scalar.activation(sgA[:], gpA[:], SIG)
                nc.scalar.activation(sgB[64:128, :], gpB[64:128, :], TANH)
                nc.scalar.activation(sgB[0:64, :], gpB[0:64, :], SIG)

                t1 = work.tile([64, LPC], F32, tag="t1")
                nc.vector.tensor_mul(t1[:], sgA[0:64, :], cst[:])
                t2 = work.tile([64, LPC], F32, tag="t2")
                nc.vector.tensor_mul(t2[:], sgA[64:128, :], sgB[64:128, :])
                nc.vector.tensor_add(cst[:], t1[:], t2[:])

                tct = work.tile([64, LPC], F32, tag="tct")
                nc.scalar.activation(tct[:], cst[:], TANH)

                # h = sigmoid(o) * tanh(c); on the round's last tick fold in
                # the next round's fill mask so downstream state starts at 0
                if t == C - 1 and r + 1 <= FILL_MAX:
                    nc.vector.tensor_scalar_mul(tct[:], tct[:], mcol[:, r + 1:r + 2])
                # write the shifted y half directly (same product as h), then
                # the h half: the next burst then has a single gate (hmul)
                # instead of hmul->shift, avoiding a mid-burst PE restart
                nc.vector.tensor_mul(lhsT[0:64, 1:LPC], sgB[0:64, 0:LPC - 1],
                                     tct[:, 0:LPC - 1])
                nc.vector.tensor_mul(lhsT[64:128, :], sgB[0:64, :], tct[:])
                nc.vector.tensor_copy(outch[:, t:t + 1], lhsT[64:128, LPC - 1:LPC])

            # accumulate the final h on the one round where it's this core's
            if r >= R - 1:
                nc.vector.scalar_tensor_tensor(
                    fhw[64:128, :], lhsT[64:128, :], fcolw[64:128, r:r + 1],
                    fhw[64:128, :], op0=MUL_OP, op1=ADD_OP,
                )

            if r + 1 < ROUNDS:
                slots_cur = slots_next

            # ship this round's boundary chunk (consumed at round r+S)
            if r < NCC:
                nc.scalar.dma_start(out=ccin[r][:], in_=outch[:])
                nc.gpsimd.collective_compute(
                    "AllGather", mybir.AluOpType.bypass, replica_groups=groups,
                    ins=[ccin[r][:]], outs=[agout[r][:]],
                )

        # ---- head: gather final h, logits, softmax over layers, argmax ----
        nc.sync.dma_start(out=hfin[:], in_=fhw[64:128, :])
        nc.gpsimd.collective_compute(
            "AllGather", mybir.AluOpType.bypass, replica_groups=groups,
            ins=[hfin[:]], outs=[hfall[:]],
        )
        HT = state.tile([64, NL], HDT)
        nc.sync.dma_start(
            out=HT.rearrange("p (s i) -> p s i", i=LPC),
            in_=hfall[:].rearrange("s p i -> p s i"),
        )
        logp = gpool.tile([V, NL], F32, tag="logp", bufs=1)
        nc.tensor.matmul(logp[:], wfct[:], HT[:], start=True, stop=True)
        logits = work.tile([V, NL], F32, tag="logits")
        nc.scalar.add(logits[:], logp[:], bfct[:, 0:1])

        mx = work.tile([V, 1], F32, tag="mx")
        nc.vector.tensor_reduce(
            out=mx[:], in_=logits[:], axis=mybir.AxisListType.X, op=mybir.AluOpType.max)
        nmx = work.tile([V, 1], F32, tag="nmx")
        nc.scalar.mul(nmx[:], mx[:], -1.0)
        ex = work.tile([V, NL], F32, tag="ex")
        nc.scalar.activation(
            ex[:], logits[:], mybir.ActivationFunctionType.Exp, bias=nmx[:, 0:1])
        sm = work.tile([V, 1], F32, tag="sm")
        nc.vector.tensor_reduce(
            out=sm[:], in_=ex[:], axis=mybir.AxisListType.X, op=ADD_OP)
        rsm = work.tile([V, 1], F32, tag="rsm")
        nc.vector.reciprocal(rsm[:], sm[:])
        probs = work.tile([V, NL], F32, tag="probs")
        nc.scalar.mul(probs[:], ex[:], rsm[:, 0:1])

        tp = gpool.tile([128, V], F32, tag="tp", bufs=1)
        nc.tensor.transpose(tp[:], probs[:], idn[:])
        m2 = work.tile([128, 1], F32, tag="m2")
        nc.vector.tensor_reduce(
            out=m2[:], in_=tp[:], axis=mybir.AxisListType.X, op=mybir.AluOpType.max)
        m2b = work.tile([128, V], F32, tag="m2b")
        nc.scalar.mul(m2b[:], onesv[:], m2[:, 0:1])
        eq = work.tile([128, V], F32, tag="eq")
        nc.vector.tensor_tensor(eq[:], tp[:], m2b[:], op=mybir.AluOpType.is_equal)
        val = work.tile([128, V], F32, tag="val")
        nc.vector.tensor_mul(val[:], eq[:], iotar[:])
        mr = work.tile([128, 1], F32, tag="mr")
        nc.vector.tensor_reduce(
            out=mr[:], in_=val[:], axis=mybir.AxisListType.X, op=mybir.AluOpType.max)
        idx = work.tile([128, 1], F32, tag="idx")
        nc.vector.tensor_sub(idx[:], vct[:], mr[:])
        nc.scalar.dma_start(out=out_d[:], in_=idx[:])

    nc.finalize()
    return nc


def _prep_in_maps(inputs):
    x = np.asarray(inputs["x"]).astype(np.int64)
    embed = np.asarray(inputs["embed"], dtype=np.float32)
    xe = embed[x, 0]  # (T,)

    Wih_full = np.zeros((NL, 4 * H, H), np.float32)
    Wih_full[0, :, 0] = np.asarray(inputs["Wih0"], np.float32)[:, 0]
    Wih_full[1:] = np.asarray(inputs["Wih"], np.float32)
    Whh_full = np.concatenate(
        [np.asarray(inputs["Whh0"], np.float32)[None],
         np.asarray(inputs["Whh"], np.float32)], axis=0)
    b_full = np.concatenate(
        [(np.asarray(inputs["bih0"], np.float32)
          + np.asarray(inputs["bhh0"], np.float32))[None],
         np.asarray(inputs["bih"], np.float32)
         + np.asarray(inputs["bhh"], np.float32)], axis=0)  # (NL, 256)

    Wcat = np.concatenate([Wih_full, Whh_full], axis=2)      # (NL, 256, 128)
    # permute pytorch gate order [i f g o] -> [f i o g] so the device layout
    # has half-A rows = [f; i] and half-B rows = [o; g]
    perm = np.r_[64:128, 0:64, 192:256, 128:192]
    Wcat = Wcat[:, perm, :]
    b_full = b_full[:, perm]
    lhsT_all = np.ascontiguousarray(np.transpose(Wcat, (0, 2, 1)))  # (NL,128,256)

    wfct = np.asarray(inputs["Wfc"], np.float32).T.astype(HDT_NP)  # (64, V)
    bfc = np.asarray(inputs["bfc"], np.float32).reshape(V, 1)
    iotar = np.broadcast_to(
        (V - np.arange(V, dtype=np.float32))[None, :], (128, V)).copy()
    idn = np.eye(V, dtype=np.float32)
    aginit = np.zeros((NCORE, 64, C), HDT_NP)

    in_maps = []
    for k in range(NCORE):
        lhsT_k = lhsT_all[k * LPC:(k + 1) * LPC]  # (LPC, 128, 256)
        wts = (lhsT_k.reshape(LPC, 128, 2, 128)
               .transpose(0, 2, 1, 3)
               .reshape(2 * LPC, 128, 128).astype(HDT_NP))
        wts = np.ascontiguousarray(wts.transpose(1, 0, 2)).reshape(128, 2 * LPC * 128)
        bT = np.ascontiguousarray(b_full[k * LPC:(k + 1) * LPC].astype(np.float32))  # (LPC, 256)

        selm = np.zeros((64, C, NCORE), np.float32)
        if k > 0:
            selm[:, :, k - 1] = 1.0
        xest = np.zeros((64, ROUNDS * C), np.float32)
        if k == 0:
            xest[0, :T] = xe
        mcol = np.ones((64, ROUNDS + 1), np.float32)
        mcol[:, :S * k + 1] = 0.0  # zero state at start of rounds <= S*k
        fcol = np.zeros((64, ROUNDS), HDT_NP)
        fcol[:, S * k + R - 1] = 1.0

        in_maps.append({
            "wts": wts,
            "bT": bT,
            "idl": np.eye(LPC, dtype=np.float32),
            "selm": selm.reshape(64, C * NCORE),
            "xest": xest,
            "mcol": mcol,
            "fcol": fcol,
            "onesv": np.ones((128, V), np.float32),
            "vct": np.full((128, 1), float(V), np.float32),
            "wfct": wfct,
            "bfc": bfc,
            "iotar": iotar,
            "idn": idn,
            "aginit": aginit,
        })
    return in_maps


def _run(inputs, trace=False):
    if "nc" not in _CACHE:
        _CACHE["nc"] = _build()
    nc = _CACHE["nc"]
    in_maps = _prep_in_maps(inputs)
    res = run_bass_kernel_spmd(nc, in_maps, list(range(NCORE)), trace=trace)
    out = np.asarray(res.results[0]["out_idx"], np.float32).reshape(NL)
    idx = np.rint(out).astype(np.int32)
    return idx, res


def kernel(**inputs) -> np.ndarray:
    idx, _ = _run(inputs, trace=False)
    return idx
